# revision 33
# baseline (speedup 1.0000x reference)
"""MoE routing kernel for 8 Trainium2 NeuronCores (Bass/Tile, SPMD).

Strategy (expert-parallel, matching the sharding hint):
  - Host computes the gate (softmax + top-2) and dispatches tokens: each of
    the 8 cores owns 2 of the 16 routed experts and receives only the tokens
    routed to its experts (gathered + transposed + zero-padded to a slot
    capacity).  Experts are paired hot+cold across cores so the two slot
    capacities (cap1 >= cap2) are as small as possible.
  - The output layer (ow) is linear and commutes with the weighted combine,
    so it is folded into each expert's second matmul on the host
    (w2ot = w2[e].T @ ow.T), shrinking stage-2 work by W/OUT = 4x.
  - The shared expert is sharded over its intermediate dim (2048/8=256 rows
    per core); every core computes a partial for all 2048 tokens, also with
    ow folded in.  Bias terms that commute with the output layer
    (b2, sb2, ob) are applied analytically on the host.
  - Schedule: the shared expert runs FIRST (its inputs are small, so the PE
    starts almost immediately) while the bulky routed-expert weights stream
    in behind it; the routed experts then run back-to-back.  All DMAs are
    emitted in first-use order across multiple queues.
  - Host combines: scatter-add of combine-weight-scaled routed partials +
    shared partials + analytic bias terms.
"""
import sys

if "/opt/trn_rl_repo" not in sys.path:
    sys.path.insert(0, "/opt/trn_rl_repo")

import numpy as np
import concourse.bass as bass
import concourse.tile as tile
from concourse import mybir
from concourse.bass_utils import run_bass_kernel_spmd

B = 2048
W = 512
E = 16
TOPK = 2
INTER = 1024
SH = 2048
OUT = 128
NCORES = 8
EPC = E // NCORES          # expert slots per core = 2
SHS = SH // NCORES         # shared-expert inter slice per core = 256
KW = W // 128              # k-tiles over W = 4
MI = INTER // 128          # m-tiles over INTER = 8
MS = SHS // 128            # m-tiles over shared slice = 2
CHUNKS = (512, 512, 512, 512)        # shared-expert token chunks (sum = B)
assert sum(CHUNKS) == B
F32 = mybir.dt.float32
F16 = mybir.dt.float16
DT = F16                   # device datapath dtype for matmul operands
NPDT = np.float16

# set by test.py to collect a profile; results stashed in LAST_RESULTS
TRACE = False
TRACE_KW = {}
LAST_RESULTS = None


def _legalize_waits(nc):
    """This container's walrus accepts at most 1 sync wait per instruction
    (2 for EventSemaphore).  Hoist excess waits emitted by the Tile
    scheduler into standalone EventSemaphore instructions."""
    for fn in nc.m.functions:
        for blk in fn.blocks:
            out = []
            changed = False
            for inst in blk.instructions:
                si = getattr(inst, "sync_info", None)
                waits = list(si.on_wait) if si is not None and si.on_wait else []
                cap = 2 if isinstance(inst, mybir.InstEventSemaphore) else 1
                if len(waits) > cap:
                    extra, keep = waits[:-cap], waits[-cap:]
                    for i in range(0, len(extra), 2):
                        out.append(mybir.InstEventSemaphore(
                            name=nc.get_next_instruction_name(),
                            engine=inst.engine,
                            ins=[], outs=[],
                            sync_info=mybir.SyncInfo(
                                on_wait=list(extra[i:i + 2]), on_update=[]),
                        ))
                    si.on_wait = keep
                    changed = True
                out.append(inst)
            if changed:
                blk.instructions = out


def _strip_end_clears(nc):
    """Drop the end-of-program semaphore RANGE_CLEAR sweep (~6.5us of
    serial semaphore resets) plus the post-clear drain/barrier round.  The
    clears only prepare device state for re-executing the same loaded NEFF,
    which never happens here (each kernel() call builds a fresh program).
    The DMA-completion waits and the first drain/barrier round are kept, so
    outputs are still guaranteed complete at program end."""
    blk = nc.m.functions[0].blocks[-1]
    isa_idx = [i for i, inst in enumerate(blk.instructions)
               if isinstance(inst, mybir.InstISA)]
    if isa_idx:
        cut = isa_idx[0]
        if cut > 0 and isinstance(blk.instructions[cut - 1], mybir.InstDrain):
            cut -= 1
        blk.instructions = blk.instructions[:cut]
    # The remaining per-engine Drains carry the kernel semaphore range and
    # are expanded by the backend into the per-semaphore reset sweep; drop
    # them too (their sync_info waits/updates move to a plain EventSemaphore
    # so the barrier chain stays intact).
    out = []
    for inst in blk.instructions:
        if isinstance(inst, mybir.InstDrain):
            si = getattr(inst, "sync_info", None)
            if si is not None and (si.on_wait or si.on_update):
                out.append(mybir.InstEventSemaphore(
                    name=nc.get_next_instruction_name(),
                    engine=inst.engine, ins=[], outs=[],
                    sync_info=mybir.SyncInfo(
                        on_wait=list(si.on_wait or []),
                        on_update=list(si.on_update or [])),
                ))
            continue
        out.append(inst)
    blk.instructions = out


def _strip_main_barrier(nc):
    """Remove the all-engine barrier at the end of the preamble (main)
    block.  It only guards the Bass-internal const memsets, which nothing
    in this kernel reads; removing it lets each engine fall through to its
    first DMA trigger ~1us earlier."""
    main = nc.m.functions[0].blocks[0]
    main.instructions = [
        inst for inst in main.instructions
        if not (isinstance(inst, mybir.InstDrain)
                or (isinstance(inst, mybir.InstEventSemaphore)
                    and str(getattr(inst, "name", "")).startswith("barrier_")))
    ]


def _hoist_first_dmas(nc):
    """Move the first wait-free DMA trigger of the Activation and SP engines
    from the body block into the main (preamble) block, just before each
    engine's branch, so they fire during fixed program boot."""
    fn = nc.m.functions[0]
    main, body = fn.blocks[0], fn.blocks[1]
    for eng in (mybir.EngineType.Activation, mybir.EngineType.SP):
        moved = None
        for inst in body.instructions:
            if getattr(inst, "engine", None) != eng:
                continue
            if isinstance(inst, mybir.InstDMACopy):
                si = getattr(inst, "sync_info", None)
                if si is None or not si.on_wait:
                    moved = inst
            break   # only consider the engine's first body instruction
        if moved is None:
            continue
        body.instructions.remove(moved)
        br = next(i for i, ins in enumerate(main.instructions)
                  if isinstance(ins, mybir.InstUnconditionalBranch)
                  and ins.engine == eng)
        main.instructions.insert(br, moved)


def _merge_end_block(nc):
    """Append the (now tiny) end block's instructions to the body block and
    drop the separate block, removing the branch + its ~16KB end-of-program
    iram fetch from the critical path."""
    fn = nc.m.functions[0]
    if len(fn.blocks) < 3:
        return
    body, end = fn.blocks[-2], fn.blocks[-1]
    body.instructions = [
        inst for inst in body.instructions
        if not isinstance(inst, mybir.InstUnconditionalBranch)
    ] + list(end.instructions)
    fn.blocks.pop()


def _build_nc(caps, legalize=True):
    """Build the SPMD Bass program for per-slot token capacities
    `caps = (cap1, cap2)` (multiples of 16, each <= 512)."""
    nc = bass.Bass("TRN2", target_bir_lowering=False, debug=False)
    capsum = sum(caps)

    def din(name, f, dt=DT):
        return nc.dram_tensor(name, [128, f], dt, kind="ExternalInput").ap()

    # shared expert + all tokens (needed first)
    bias = din("bias", EPC * 2 * MI + 2 * MS, F32)  # b1/b3 per slot, sb1/sb3
    sw1t = din("sw1t", KW * SHS)             # shared slice: sw1[s].T packed k-blocks
    sw3t = din("sw3t", KW * SHS)
    sw2ot = din("sw2ot", MS * OUT)           # (sw2[:, s].T @ ow.T) packed
    xtc = din("xtc", B * KW)                 # x.T packed per chunk: chunk c at
    #   cols [c*KW*CH, (c+1)*KW*CH), k-block k at [c*KW*CH + k*CH, ... + CH)
    # routed expert slots
    w1s = [din(f"w1t{j}", KW * INTER) for j in range(EPC)]
    w3s = [din(f"w3t{j}", KW * INTER) for j in range(EPC)]
    xgs = [din(f"xg{j}", KW * caps[j]) for j in range(EPC)]
    w2s = [din(f"w2ot{j}", MI * OUT) for j in range(EPC)]

    yr = nc.dram_tensor("yr", [128, capsum], F16, kind="ExternalOutput").ap()
    zt = nc.dram_tensor("zt", [128, B], F16, kind="ExternalOutput").ap()

    LR = mybir.ActivationFunctionType.Lrelu

    with tile.TileContext(nc) as tc:
        with tc.tile_pool(name="wts", bufs=1) as wts, \
             tc.tile_pool(name="work", bufs=3) as work, \
             tc.tile_pool(name="hts", bufs=1) as hts, \
             tc.tile_pool(name="outs", bufs=2) as outs, \
             tc.tile_pool(name="ps", bufs=2, space="PSUM") as ps:

            # ---- DMAs in first-use order, spread across queues.  Tile
            # dependencies are whole-tile, so the first two chunks of x are
            # split into half-K tiles: the PE's first matmul only needs
            # sw1 (256KB) + the first half-chunk (256KB).  DMA triggers cost
            # ~600ns of sequencer time each, so pieces are kept >= 256KB.
            sw1_t = wts.tile([128, KW * SHS], DT, tag="sw1")
            nc.scalar.dma_start(sw1_t[:], sw1t[:])
            sw3_t = wts.tile([128, KW * SHS], DT, tag="sw3")
            nc.scalar.dma_start(sw3_t[:], sw3t[:])
            bias_t = wts.tile([128, bias.shape[1]], F32, tag="bias")
            nc.scalar.dma_start(bias_t[:], bias[:])
            sw2_t = wts.tile([128, MS * OUT], DT, tag="sw2")
            nc.scalar.dma_start(sw2_t[:], sw2ot[:])

            def sw3_ap(k, m):
                return sw3_t[:, (k * MS + m) * 128:(k * MS + m + 1) * 128]
            # xt chunk tiles; chunks 0 and 1 as two half-K tiles each
            xt_pieces = []   # per chunk: list of (tile, k_lo, k_hi)
            off = 0
            for c, ch in enumerate(CHUNKS):
                if c < 2:
                    pieces = []
                    for h, (klo, khi) in enumerate(((0, KW // 2), (KW // 2, KW))):
                        t = wts.tile([128, (khi - klo) * ch], DT,
                                     tag=f"xtc{c}h{h}", name=f"xtc{c}h{h}")
                        nc.sync.dma_start(
                            t[:], xtc[:, off * KW + klo * ch: off * KW + khi * ch])
                        pieces.append((t, klo, khi))
                    xt_pieces.append(pieces)
                else:
                    t = wts.tile([128, KW * ch], DT, tag=f"xtc{c}", name=f"xtc{c}")
                    nc.sync.dma_start(t[:], xtc[:, off * KW:(off + ch) * KW])
                    xt_pieces.append([(t, 0, KW)])
                off += ch

            def xt_ap(c, k, ch):
                for t, klo, khi in xt_pieces[c]:
                    if klo <= k < khi:
                        return t[:, (k - klo) * ch:(k - klo + 1) * ch]
                raise AssertionError
            # expert slots stream behind the shared phase
            w1_ts, w3_ts, xg_ts, w2_ts = [], [], [], []
            for j in range(EPC):
                eng = nc.scalar if j == 0 else nc.sync
                t = wts.tile([128, KW * INTER], DT, tag=f"w1s{j}")
                eng.dma_start(t[:], w1s[j][:])
                w1_ts.append(t)
                t = wts.tile([128, KW * caps[j]], DT, tag=f"xgs{j}")
                nc.gpsimd.dma_start(t[:], xgs[j][:])
                xg_ts.append(t)
                t = wts.tile([128, KW * INTER], DT, tag=f"w3s{j}")
                eng.dma_start(t[:], w3s[j][:])
                w3_ts.append(t)
                t = wts.tile([128, MI * OUT], DT, tag=f"w2s{j}")
                nc.gpsimd.dma_start(t[:], w2s[j][:])
                w2_ts.append(t)

            # ---- activation-table preload: a tiny LRELU on a memset tile so
            # the first real LEAKY_RELU doesn't pay ACT_TABLE_LOAD.  Emitted
            # after the DMA triggers so it doesn't delay them.
            warm = wts.tile([128, 1], F32, tag="warm")
            nc.gpsimd.memset(warm[:], 0.0)
            warm2 = wts.tile([128, 1], DT, tag="warm2")
            nc.scalar.activation(warm2[:], warm[:], LR, alpha=0.01)

            def b_ap(col):  # [128,1] per-partition bias column
                return bias_t[:, col:col + 1]

            def epilogue(p1, p3, bcol1, bcol3, csz, htag):
                """h = lrelu(p1 + b1) * (p3 + b3), split over 3 engines."""
                a = work.tile([128, csz], DT, tag="act_a")
                nc.scalar.activation(a[:], p1[:], LR, bias=b_ap(bcol1), alpha=0.01)
                t3 = work.tile([128, csz], DT, tag="act_b")
                nc.vector.tensor_scalar_add(t3[:], p3[:], b_ap(bcol3))
                ht = hts.tile([128, csz], DT, tag=htag, bufs=3)
                nc.vector.tensor_mul(ht[:], a[:], t3[:])
                return ht

            # ---- shared expert: token chunks, stage-2 trails by one chunk
            # so the in-order PE stream never waits on the DVE.
            pend = None

            def flush_shared(pz, t0, ch, hs_list):
                for m, hs in hs_list:
                    nc.tensor.matmul(pz[:], sw2_t[:, m * OUT:(m + 1) * OUT],
                                     hs[:], start=(m == 0), stop=(m == MS - 1))
                zo = outs.tile([128, ch], F16, tag="zo")
                nc.vector.tensor_copy(zo[:], pz[:])
                nc.sync.dma_start(zt[:, t0:t0 + ch], zo[:])

            t0 = 0
            for c, ch in enumerate(CHUNKS):
                pz = ps.tile([128, ch], F32, tag="py")
                hs_list = []
                if c == 0:
                    # first chunk: run BOTH m-tiles' p1 matmuls before any
                    # p3, so the PE has ~2us of sw1-only work while the sw3
                    # tile is still streaming in behind sw1 on its queue.
                    p1s, p3s = [], []
                    for m in range(MS):
                        p1 = ps.tile([128, ch], F32, tag="p1", bufs=3)
                        for k in range(KW):
                            nc.tensor.matmul(p1[:], sw1_t[:, (k * MS + m) * 128:(k * MS + m + 1) * 128],
                                             xt_ap(c, k, ch),
                                             start=(k == 0), stop=(k == KW - 1))
                        p1s.append(p1)
                    for m in range(MS):
                        p3 = ps.tile([128, ch], F32, tag="p3", bufs=3)
                        for k in range(KW):
                            nc.tensor.matmul(p3[:], sw3_ap(k, m), xt_ap(c, k, ch),
                                             start=(k == 0), stop=(k == KW - 1))
                        p3s.append(p3)
                    for m in range(MS):
                        hs = epilogue(p1s[m], p3s[m], EPC * 2 * MI + m,
                                      EPC * 2 * MI + MS + m, ch, f"hs{m}")
                        hs_list.append((m, hs))
                else:
                    for m in range(MS):
                        p1 = ps.tile([128, ch], F32, tag="p1", bufs=3)
                        p3 = ps.tile([128, ch], F32, tag="p3", bufs=3)
                        for k in range(KW):
                            nc.tensor.matmul(p1[:], sw1_t[:, (k * MS + m) * 128:(k * MS + m + 1) * 128],
                                             xt_ap(c, k, ch),
                                             start=(k == 0), stop=(k == KW - 1))
                        for k in range(KW):
                            nc.tensor.matmul(p3[:], sw3_ap(k, m), xt_ap(c, k, ch),
                                             start=(k == 0), stop=(k == KW - 1))
                        hs = epilogue(p1, p3, EPC * 2 * MI + m, EPC * 2 * MI + MS + m,
                                      ch, f"hs{m}")
                        hs_list.append((m, hs))
                if pend is not None:
                    flush_shared(*pend)
                pend = (pz, t0, ch, hs_list)
                t0 += ch
            flush_shared(*pend)

            # ---- routed experts: stage-2 trails stage-1 by LAG m-tiles.
            LAG = 2
            for j in range(EPC):
                cap = caps[j]
                py = ps.tile([128, cap], F32, tag="py")
                hts_pend = []
                for m in range(MI):
                    p1 = ps.tile([128, cap], F32, tag="p1", bufs=3)
                    p3 = ps.tile([128, cap], F32, tag="p3", bufs=3)
                    for k in range(KW):
                        nc.tensor.matmul(p1[:], w1_ts[j][:, (k * MI + m) * 128:(k * MI + m + 1) * 128],
                                         xg_ts[j][:, k * cap:(k + 1) * cap],
                                         start=(k == 0), stop=(k == KW - 1))
                    for k in range(KW):
                        nc.tensor.matmul(p3[:], w3_ts[j][:, (k * MI + m) * 128:(k * MI + m + 1) * 128],
                                         xg_ts[j][:, k * cap:(k + 1) * cap],
                                         start=(k == 0), stop=(k == KW - 1))
                    ht = epilogue(p1, p3, j * 2 * MI + m, j * 2 * MI + MI + m,
                                  cap, f"ht{m}")
                    hts_pend.append((m, ht))
                    if len(hts_pend) > LAG:
                        md, htd = hts_pend.pop(0)
                        nc.tensor.matmul(py[:], w2_ts[j][:, md * OUT:(md + 1) * OUT],
                                         htd[:], start=(md == 0), stop=(md == MI - 1))
                for md, htd in hts_pend:
                    nc.tensor.matmul(py[:], w2_ts[j][:, md * OUT:(md + 1) * OUT],
                                     htd[:], start=(md == 0), stop=(md == MI - 1))
                yo = outs.tile([128, cap], F16, tag="yo")
                nc.vector.tensor_copy(yo[:], py[:])
                off = sum(caps[:j])
                nc.sync.dma_start(yr[:, off:off + cap], yo[:])

    _strip_end_clears(nc)
    _strip_main_barrier(nc)
    _hoist_first_dmas(nc)
    _merge_end_block(nc)
    if legalize:
        _legalize_waits(nc)
    return nc


_NC_CACHE = {}


def _pack_kblocks(mat):
    """[Ktot, F] -> [128, (Ktot/128)*F] with col block k = mat[128k:128(k+1), :]."""
    ktot, f = mat.shape
    assert ktot % 128 == 0
    return np.ascontiguousarray(
        mat.reshape(ktot // 128, 128, f).transpose(1, 0, 2).reshape(128, -1))


def _ceil16(n):
    return max(128, -(-n // 16) * 16)


def prepare(x, task_id, gate_w, w1, b1, w2, b2, w3, b3,
            sw1, sb1, sw2, sb2, sw3, sb3, ow, ob):
    """Host-side routing + packing.  Returns everything needed to launch the
    device program and combine its partial outputs."""
    x = np.asarray(x, np.float32)
    f32 = lambda a: np.asarray(a, np.float32)
    gate_w, w1, b1, w2, b2, w3, b3 = map(f32, (gate_w, w1, b1, w2, b2, w3, b3))
    sw1, sb1, sw2, sb2, sw3, sb3, ow, ob = map(f32, (sw1, sb1, sw2, sb2, sw3, sb3, ow, ob))

    # ---- host gate: softmax + top-2 (the routing decision) ----
    logits = x @ gate_w.T
    logits -= logits.max(axis=1, keepdims=True)
    ex = np.exp(logits)
    scores = ex / ex.sum(axis=1, keepdims=True)            # [B, E] fp32
    order = np.argsort(-scores, axis=1, kind="stable")[:, :TOPK]   # [B, 2]

    tok_lists = []
    for e in range(E):
        sel = np.nonzero((order == e).any(axis=1))[0]
        tok_lists.append(sel)

    # hot+cold pairing: core i gets (rank i, rank 15-i) by token count
    rank = sorted(range(E), key=lambda e: -len(tok_lists[e]))
    slot_exp = [[rank[i], rank[E - 1 - i]] for i in range(NCORES)]
    caps = tuple(_ceil16(max(len(tok_lists[slot_exp[c][j]]) for c in range(NCORES)))
                 for j in range(EPC))

    if caps not in _NC_CACHE:
        _NC_CACHE[caps] = _build_nc(caps)
    nc = _NC_CACHE[caps]

    # ---- pack per-core inputs (device datapath dtype) ----
    # xtc: chunk-major, then k-block: [128, B * KW]
    xt_k = x.T.reshape(KW, 128, B)                         # [k, p, t]
    blocks = []
    off = 0
    for ch in CHUNKS:
        for k in range(KW):
            blocks.append(xt_k[k, :, off:off + ch])
        off += ch
    xtc = np.ascontiguousarray(np.concatenate(blocks, axis=1)).astype(NPDT)
    in_maps = []
    for c in range(NCORES):
        m = {"xtc": xtc}
        bias_cols = []
        for j in range(EPC):
            e = slot_exp[c][j]
            cap = caps[j]
            toks = tok_lists[e]
            xge = np.zeros((W, cap), np.float32)
            xge[:, :len(toks)] = x[toks].T
            m[f"xg{j}"] = _pack_kblocks(xge).astype(NPDT)
            m[f"w1t{j}"] = _pack_kblocks(w1[e].T.copy()).astype(NPDT)
            m[f"w3t{j}"] = _pack_kblocks(w3[e].T.copy()).astype(NPDT)
            m[f"w2ot{j}"] = _pack_kblocks(w2[e].T @ ow.T).astype(NPDT)
        for j in range(EPC):
            e = slot_exp[c][j]
            bias_cols.append(b1[e].reshape(MI, 128).T)     # [128, MI]
            bias_cols.append(b3[e].reshape(MI, 128).T)
        s = slice(c * SHS, (c + 1) * SHS)
        bias_cols.append(sb1[s].reshape(MS, 128).T)
        bias_cols.append(sb3[s].reshape(MS, 128).T)
        m["bias"] = np.ascontiguousarray(np.concatenate(bias_cols, axis=1))
        m["sw1t"] = _pack_kblocks(sw1[s].T.copy()).astype(NPDT)
        m["sw3t"] = _pack_kblocks(sw3[s].T.copy()).astype(NPDT)
        m["sw2ot"] = _pack_kblocks(sw2[:, s].T @ ow.T).astype(NPDT)
        in_maps.append(m)

    # dense combine weights [B, E] (zero except the top-2 experts per token)
    combine_w = np.zeros((B, E), np.float32)
    rows = np.arange(B)
    combine_w[rows[:, None], order] = np.take_along_axis(scores, order, axis=1)
    # analytic bias terms: sum_e combine[:,e] * (b2[e] @ ow.T)  +  sb2 @ ow.T + ob
    base = combine_w @ (b2 @ ow.T) + sb2 @ ow.T + ob

    return dict(nc=nc, caps=caps, slot_exp=slot_exp, in_maps=in_maps,
                tok_lists=tok_lists, combine_w=combine_w, base=base)


def combine(p, results):
    """Combine per-core device partials into the full [B, OUT] output."""
    caps, slot_exp, tok_lists, combine_w = (
        p["caps"], p["slot_exp"], p["tok_lists"], p["combine_w"])
    out = p["base"].astype(np.float32).copy()
    for c in range(NCORES):
        r = results[c]
        out += r["zt"].astype(np.float32).T
        for j in range(EPC):
            e = slot_exp[c][j]
            toks = tok_lists[e]
            off = sum(caps[:j])
            yre = r["yr"][:, off:off + len(toks)].astype(np.float32)  # [OUT, cnt]
            out[toks] += combine_w[toks, e][:, None] * yre.T
    return out


def kernel(x, task_id, gate_w, w1, b1, w2, b2, w3, b3,
           sw1, sb1, sw2, sb2, sw3, sb3, ow, ob):
    global LAST_RESULTS
    p = prepare(x, task_id, gate_w, w1, b1, w2, b2, w3, b3,
                sw1, sb1, sw2, sb2, sw3, sb3, ow, ob)
    res = run_bass_kernel_spmd(
        p["nc"], p["in_maps"], core_ids=list(range(NCORES)),
        trace=TRACE, **TRACE_KW)
    LAST_RESULTS = res
    return combine(p, res.results)


# revision 35
# speedup vs baseline: 1.0137x; 1.0137x over previous
"""MoE routing kernel for 8 Trainium2 NeuronCores (Bass/Tile, SPMD).

Strategy (expert-parallel, matching the sharding hint):
  - Host computes the gate (softmax + top-2) and dispatches tokens: each of
    the 8 cores owns 2 of the 16 routed experts and receives only the tokens
    routed to its experts (gathered + transposed + zero-padded to a slot
    capacity).  Experts are paired hot+cold across cores so the two slot
    capacities (cap1 >= cap2) are as small as possible.
  - The output layer (ow) is linear and commutes with the weighted combine,
    so it is folded into each expert's second matmul on the host
    (w2ot = w2[e].T @ ow.T), shrinking stage-2 work by W/OUT = 4x.
  - The shared expert is sharded over its intermediate dim (2048/8=256 rows
    per core); every core computes a partial for all 2048 tokens, also with
    ow folded in.  Bias terms that commute with the output layer
    (b2, sb2, ob) are applied analytically on the host.
  - Schedule: the shared expert runs FIRST (its inputs are small, so the PE
    starts almost immediately) while the bulky routed-expert weights stream
    in behind it; the routed experts then run back-to-back.  All DMAs are
    emitted in first-use order across multiple queues.
  - Host combines: scatter-add of combine-weight-scaled routed partials +
    shared partials + analytic bias terms.
"""
import sys

if "/opt/trn_rl_repo" not in sys.path:
    sys.path.insert(0, "/opt/trn_rl_repo")

import numpy as np
import concourse.bass as bass
import concourse.tile as tile
from concourse import mybir
from concourse.bass_utils import run_bass_kernel_spmd

B = 2048
W = 512
E = 16
TOPK = 2
INTER = 1024
SH = 2048
OUT = 128
NCORES = 8
EPC = E // NCORES          # expert slots per core = 2
SHS = SH // NCORES         # shared-expert inter slice per core = 256
KW = W // 128              # k-tiles over W = 4
MI = INTER // 128          # m-tiles over INTER = 8
MS = SHS // 128            # m-tiles over shared slice = 2
CHUNKS = (512, 512, 512, 512)        # shared-expert token chunks (sum = B)
assert sum(CHUNKS) == B
F32 = mybir.dt.float32
F16 = mybir.dt.float16
DT = F16                   # device datapath dtype for matmul operands
NPDT = np.float16

# set by test.py to collect a profile; results stashed in LAST_RESULTS
TRACE = False
TRACE_KW = {}
LAST_RESULTS = None


def _legalize_waits(nc):
    """This container's walrus accepts at most 1 sync wait per instruction
    (2 for EventSemaphore).  Hoist excess waits emitted by the Tile
    scheduler into standalone EventSemaphore instructions."""
    for fn in nc.m.functions:
        for blk in fn.blocks:
            out = []
            changed = False
            for inst in blk.instructions:
                si = getattr(inst, "sync_info", None)
                waits = list(si.on_wait) if si is not None and si.on_wait else []
                cap = 2 if isinstance(inst, mybir.InstEventSemaphore) else 1
                if len(waits) > cap:
                    extra, keep = waits[:-cap], waits[-cap:]
                    for i in range(0, len(extra), 2):
                        out.append(mybir.InstEventSemaphore(
                            name=nc.get_next_instruction_name(),
                            engine=inst.engine,
                            ins=[], outs=[],
                            sync_info=mybir.SyncInfo(
                                on_wait=list(extra[i:i + 2]), on_update=[]),
                        ))
                    si.on_wait = keep
                    changed = True
                out.append(inst)
            if changed:
                blk.instructions = out


def _strip_end_clears(nc):
    """Drop the end-of-program semaphore RANGE_CLEAR sweep (~6.5us of
    serial semaphore resets) plus the post-clear drain/barrier round.  The
    clears only prepare device state for re-executing the same loaded NEFF,
    which never happens here (each kernel() call builds a fresh program).
    The DMA-completion waits and the first drain/barrier round are kept, so
    outputs are still guaranteed complete at program end."""
    blk = nc.m.functions[0].blocks[-1]
    isa_idx = [i for i, inst in enumerate(blk.instructions)
               if isinstance(inst, mybir.InstISA)]
    if isa_idx:
        cut = isa_idx[0]
        if cut > 0 and isinstance(blk.instructions[cut - 1], mybir.InstDrain):
            cut -= 1
        blk.instructions = blk.instructions[:cut]
    # The remaining per-engine Drains carry the kernel semaphore range and
    # are expanded by the backend into the per-semaphore reset sweep; drop
    # them too (their sync_info waits/updates move to a plain EventSemaphore
    # so the barrier chain stays intact).
    out = []
    for inst in blk.instructions:
        if isinstance(inst, mybir.InstDrain):
            si = getattr(inst, "sync_info", None)
            if si is not None and (si.on_wait or si.on_update):
                out.append(mybir.InstEventSemaphore(
                    name=nc.get_next_instruction_name(),
                    engine=inst.engine, ins=[], outs=[],
                    sync_info=mybir.SyncInfo(
                        on_wait=list(si.on_wait or []),
                        on_update=list(si.on_update or [])),
                ))
            continue
        out.append(inst)
    blk.instructions = out


def _merge_end_block(nc):
    """Append the (now tiny) end block's instructions to the body block and
    drop the separate block, removing the branch + its ~16KB end-of-program
    iram fetch from the critical path."""
    fn = nc.m.functions[0]
    if len(fn.blocks) < 3:
        return
    body, end = fn.blocks[-2], fn.blocks[-1]
    body.instructions = [
        inst for inst in body.instructions
        if not isinstance(inst, mybir.InstUnconditionalBranch)
    ] + list(end.instructions)
    fn.blocks.pop()


def _build_nc(caps, legalize=True):
    """Build the SPMD Bass program for per-slot token capacities
    `caps = (cap1, cap2)` (multiples of 16, each <= 512)."""
    nc = bass.Bass("TRN2", target_bir_lowering=False, debug=False)
    capsum = sum(caps)

    def din(name, f, dt=DT):
        return nc.dram_tensor(name, [128, f], dt, kind="ExternalInput").ap()

    # shared expert + all tokens (needed first)
    bias = din("bias", EPC * 2 * MI + 2 * MS, F32)  # b1/b3 per slot, sb1/sb3
    sw1t = din("sw1t", KW * SHS)             # shared slice: sw1[s].T packed k-blocks
    sw3t = din("sw3t", KW * SHS)
    sw2ot = din("sw2ot", MS * OUT)           # (sw2[:, s].T @ ow.T) packed
    xtc = din("xtc", B * KW)                 # x.T packed per chunk: chunk c at
    #   cols [c*KW*CH, (c+1)*KW*CH), k-block k at [c*KW*CH + k*CH, ... + CH)
    # routed expert slots
    w1s = [din(f"w1t{j}", KW * INTER) for j in range(EPC)]
    w3s = [din(f"w3t{j}", KW * INTER) for j in range(EPC)]
    xgs = [din(f"xg{j}", KW * caps[j]) for j in range(EPC)]
    w2s = [din(f"w2ot{j}", MI * OUT) for j in range(EPC)]

    yr = nc.dram_tensor("yr", [128, capsum], F16, kind="ExternalOutput").ap()
    zt = nc.dram_tensor("zt", [128, B], F16, kind="ExternalOutput").ap()

    LR = mybir.ActivationFunctionType.Lrelu

    with tile.TileContext(nc) as tc:
        with tc.tile_pool(name="wts", bufs=1) as wts, \
             tc.tile_pool(name="work", bufs=3) as work, \
             tc.tile_pool(name="hts", bufs=1) as hts, \
             tc.tile_pool(name="outs", bufs=2) as outs, \
             tc.tile_pool(name="ps", bufs=2, space="PSUM") as ps:

            # ---- DMAs in first-use order, spread across queues.  Tile
            # dependencies are whole-tile, so the first two chunks of x are
            # split into half-K tiles: the PE's first matmul only needs
            # sw1 (256KB) + the first half-chunk (256KB).  DMA triggers cost
            # ~600ns of sequencer time each, so pieces are kept >= 256KB.
            sw1_t = wts.tile([128, KW * SHS], DT, tag="sw1")
            nc.scalar.dma_start(sw1_t[:], sw1t[:])
            sw3_t = wts.tile([128, KW * SHS], DT, tag="sw3")
            nc.scalar.dma_start(sw3_t[:], sw3t[:])
            bias_t = wts.tile([128, bias.shape[1]], F32, tag="bias")
            nc.scalar.dma_start(bias_t[:], bias[:])
            sw2_t = wts.tile([128, MS * OUT], DT, tag="sw2")
            nc.scalar.dma_start(sw2_t[:], sw2ot[:])

            def sw3_ap(k, m):
                return sw3_t[:, (k * MS + m) * 128:(k * MS + m + 1) * 128]
            # xt chunk tiles; chunks 0 and 1 as two half-K tiles each
            xt_pieces = []   # per chunk: list of (tile, k_lo, k_hi)
            off = 0
            for c, ch in enumerate(CHUNKS):
                if c < 2:
                    pieces = []
                    for h, (klo, khi) in enumerate(((0, KW // 2), (KW // 2, KW))):
                        t = wts.tile([128, (khi - klo) * ch], DT,
                                     tag=f"xtc{c}h{h}", name=f"xtc{c}h{h}")
                        nc.sync.dma_start(
                            t[:], xtc[:, off * KW + klo * ch: off * KW + khi * ch])
                        pieces.append((t, klo, khi))
                    xt_pieces.append(pieces)
                else:
                    t = wts.tile([128, KW * ch], DT, tag=f"xtc{c}", name=f"xtc{c}")
                    nc.sync.dma_start(t[:], xtc[:, off * KW:(off + ch) * KW])
                    xt_pieces.append([(t, 0, KW)])
                off += ch

            def xt_ap(c, k, ch):
                for t, klo, khi in xt_pieces[c]:
                    if klo <= k < khi:
                        return t[:, (k - klo) * ch:(k - klo + 1) * ch]
                raise AssertionError
            # expert slots stream behind the shared phase
            w1_ts, w3_ts, xg_ts, w2_ts = [], [], [], []
            for j in range(EPC):
                eng = nc.scalar if j == 0 else nc.sync
                t = wts.tile([128, KW * INTER], DT, tag=f"w1s{j}")
                eng.dma_start(t[:], w1s[j][:])
                w1_ts.append(t)
                t = wts.tile([128, KW * caps[j]], DT, tag=f"xgs{j}")
                nc.gpsimd.dma_start(t[:], xgs[j][:])
                xg_ts.append(t)
                t = wts.tile([128, KW * INTER], DT, tag=f"w3s{j}")
                eng.dma_start(t[:], w3s[j][:])
                w3_ts.append(t)
                t = wts.tile([128, MI * OUT], DT, tag=f"w2s{j}")
                nc.gpsimd.dma_start(t[:], w2s[j][:])
                w2_ts.append(t)

            # ---- activation-table preload: a tiny LRELU on a memset tile so
            # the first real LEAKY_RELU doesn't pay ACT_TABLE_LOAD.  Emitted
            # after the DMA triggers so it doesn't delay them.
            warm = wts.tile([128, 1], F32, tag="warm")
            nc.gpsimd.memset(warm[:], 0.0)
            warm2 = wts.tile([128, 1], DT, tag="warm2")
            nc.scalar.activation(warm2[:], warm[:], LR, alpha=0.01)

            def b_ap(col):  # [128,1] per-partition bias column
                return bias_t[:, col:col + 1]

            def epilogue(p1, p3, bcol1, bcol3, csz, htag):
                """h = lrelu(p1 + b1) * (p3 + b3), split over 3 engines."""
                a = work.tile([128, csz], DT, tag="act_a")
                nc.scalar.activation(a[:], p1[:], LR, bias=b_ap(bcol1), alpha=0.01)
                t3 = work.tile([128, csz], DT, tag="act_b")
                nc.vector.tensor_scalar_add(t3[:], p3[:], b_ap(bcol3))
                ht = hts.tile([128, csz], DT, tag=htag, bufs=3)
                nc.vector.tensor_mul(ht[:], a[:], t3[:])
                return ht

            # ---- shared expert: token chunks, stage-2 trails by one chunk
            # so the in-order PE stream never waits on the DVE.
            pend = None

            def flush_shared(pz, t0, ch, hs_list):
                for m, hs in hs_list:
                    nc.tensor.matmul(pz[:], sw2_t[:, m * OUT:(m + 1) * OUT],
                                     hs[:], start=(m == 0), stop=(m == MS - 1))
                zo = outs.tile([128, ch], F16, tag="zo")
                nc.vector.tensor_copy(zo[:], pz[:])
                nc.sync.dma_start(zt[:, t0:t0 + ch], zo[:])

            t0 = 0
            for c, ch in enumerate(CHUNKS):
                pz = ps.tile([128, ch], F32, tag="py")
                hs_list = []
                if c == 0:
                    # first chunk: run BOTH m-tiles' p1 matmuls before any
                    # p3, so the PE has ~2us of sw1-only work while the sw3
                    # tile is still streaming in behind sw1 on its queue.
                    p1s, p3s = [], []
                    for m in range(MS):
                        p1 = ps.tile([128, ch], F32, tag="p1", bufs=3)
                        for k in range(KW):
                            nc.tensor.matmul(p1[:], sw1_t[:, (k * MS + m) * 128:(k * MS + m + 1) * 128],
                                             xt_ap(c, k, ch),
                                             start=(k == 0), stop=(k == KW - 1))
                        p1s.append(p1)
                    for m in range(MS):
                        p3 = ps.tile([128, ch], F32, tag="p3", bufs=3)
                        for k in range(KW):
                            nc.tensor.matmul(p3[:], sw3_ap(k, m), xt_ap(c, k, ch),
                                             start=(k == 0), stop=(k == KW - 1))
                        p3s.append(p3)
                    for m in range(MS):
                        hs = epilogue(p1s[m], p3s[m], EPC * 2 * MI + m,
                                      EPC * 2 * MI + MS + m, ch, f"hs{m}")
                        hs_list.append((m, hs))
                else:
                    for m in range(MS):
                        p1 = ps.tile([128, ch], F32, tag="p1", bufs=3)
                        p3 = ps.tile([128, ch], F32, tag="p3", bufs=3)
                        for k in range(KW):
                            nc.tensor.matmul(p1[:], sw1_t[:, (k * MS + m) * 128:(k * MS + m + 1) * 128],
                                             xt_ap(c, k, ch),
                                             start=(k == 0), stop=(k == KW - 1))
                        for k in range(KW):
                            nc.tensor.matmul(p3[:], sw3_ap(k, m), xt_ap(c, k, ch),
                                             start=(k == 0), stop=(k == KW - 1))
                        hs = epilogue(p1, p3, EPC * 2 * MI + m, EPC * 2 * MI + MS + m,
                                      ch, f"hs{m}")
                        hs_list.append((m, hs))
                if pend is not None:
                    flush_shared(*pend)
                pend = (pz, t0, ch, hs_list)
                t0 += ch
            flush_shared(*pend)

            # ---- routed experts: stage-2 trails stage-1 by LAG m-tiles.
            LAG = 2
            for j in range(EPC):
                cap = caps[j]
                py = ps.tile([128, cap], F32, tag="py")
                hts_pend = []
                for m in range(MI):
                    p1 = ps.tile([128, cap], F32, tag="p1", bufs=3)
                    p3 = ps.tile([128, cap], F32, tag="p3", bufs=3)
                    for k in range(KW):
                        nc.tensor.matmul(p1[:], w1_ts[j][:, (k * MI + m) * 128:(k * MI + m + 1) * 128],
                                         xg_ts[j][:, k * cap:(k + 1) * cap],
                                         start=(k == 0), stop=(k == KW - 1))
                    for k in range(KW):
                        nc.tensor.matmul(p3[:], w3_ts[j][:, (k * MI + m) * 128:(k * MI + m + 1) * 128],
                                         xg_ts[j][:, k * cap:(k + 1) * cap],
                                         start=(k == 0), stop=(k == KW - 1))
                    ht = epilogue(p1, p3, j * 2 * MI + m, j * 2 * MI + MI + m,
                                  cap, f"ht{m}")
                    hts_pend.append((m, ht))
                    if len(hts_pend) > LAG:
                        md, htd = hts_pend.pop(0)
                        nc.tensor.matmul(py[:], w2_ts[j][:, md * OUT:(md + 1) * OUT],
                                         htd[:], start=(md == 0), stop=(md == MI - 1))
                for md, htd in hts_pend:
                    nc.tensor.matmul(py[:], w2_ts[j][:, md * OUT:(md + 1) * OUT],
                                     htd[:], start=(md == 0), stop=(md == MI - 1))
                yo = outs.tile([128, cap], F16, tag="yo")
                nc.vector.tensor_copy(yo[:], py[:])
                off = sum(caps[:j])
                nc.sync.dma_start(yr[:, off:off + cap], yo[:])

    _strip_end_clears(nc)
    _merge_end_block(nc)
    if legalize:
        _legalize_waits(nc)
    return nc


_NC_CACHE = {}


def _pack_kblocks(mat):
    """[Ktot, F] -> [128, (Ktot/128)*F] with col block k = mat[128k:128(k+1), :]."""
    ktot, f = mat.shape
    assert ktot % 128 == 0
    return np.ascontiguousarray(
        mat.reshape(ktot // 128, 128, f).transpose(1, 0, 2).reshape(128, -1))


def _ceil16(n):
    return max(128, -(-n // 16) * 16)


def prepare(x, task_id, gate_w, w1, b1, w2, b2, w3, b3,
            sw1, sb1, sw2, sb2, sw3, sb3, ow, ob):
    """Host-side routing + packing.  Returns everything needed to launch the
    device program and combine its partial outputs."""
    x = np.asarray(x, np.float32)
    f32 = lambda a: np.asarray(a, np.float32)
    gate_w, w1, b1, w2, b2, w3, b3 = map(f32, (gate_w, w1, b1, w2, b2, w3, b3))
    sw1, sb1, sw2, sb2, sw3, sb3, ow, ob = map(f32, (sw1, sb1, sw2, sb2, sw3, sb3, ow, ob))

    # ---- host gate: softmax + top-2 (the routing decision) ----
    logits = x @ gate_w.T
    logits -= logits.max(axis=1, keepdims=True)
    ex = np.exp(logits)
    scores = ex / ex.sum(axis=1, keepdims=True)            # [B, E] fp32
    order = np.argsort(-scores, axis=1, kind="stable")[:, :TOPK]   # [B, 2]

    tok_lists = []
    for e in range(E):
        sel = np.nonzero((order == e).any(axis=1))[0]
        tok_lists.append(sel)

    # hot+cold pairing: core i gets (rank i, rank 15-i) by token count
    rank = sorted(range(E), key=lambda e: -len(tok_lists[e]))
    slot_exp = [[rank[i], rank[E - 1 - i]] for i in range(NCORES)]
    caps = tuple(_ceil16(max(len(tok_lists[slot_exp[c][j]]) for c in range(NCORES)))
                 for j in range(EPC))

    if caps not in _NC_CACHE:
        _NC_CACHE[caps] = _build_nc(caps)
    nc = _NC_CACHE[caps]

    # ---- pack per-core inputs (device datapath dtype) ----
    # xtc: chunk-major, then k-block: [128, B * KW]
    xt_k = x.T.reshape(KW, 128, B)                         # [k, p, t]
    blocks = []
    off = 0
    for ch in CHUNKS:
        for k in range(KW):
            blocks.append(xt_k[k, :, off:off + ch])
        off += ch
    xtc = np.ascontiguousarray(np.concatenate(blocks, axis=1)).astype(NPDT)
    in_maps = []
    for c in range(NCORES):
        m = {"xtc": xtc}
        bias_cols = []
        for j in range(EPC):
            e = slot_exp[c][j]
            cap = caps[j]
            toks = tok_lists[e]
            xge = np.zeros((W, cap), np.float32)
            xge[:, :len(toks)] = x[toks].T
            m[f"xg{j}"] = _pack_kblocks(xge).astype(NPDT)
            m[f"w1t{j}"] = _pack_kblocks(w1[e].T.copy()).astype(NPDT)
            m[f"w3t{j}"] = _pack_kblocks(w3[e].T.copy()).astype(NPDT)
            m[f"w2ot{j}"] = _pack_kblocks(w2[e].T @ ow.T).astype(NPDT)
        for j in range(EPC):
            e = slot_exp[c][j]
            bias_cols.append(b1[e].reshape(MI, 128).T)     # [128, MI]
            bias_cols.append(b3[e].reshape(MI, 128).T)
        s = slice(c * SHS, (c + 1) * SHS)
        bias_cols.append(sb1[s].reshape(MS, 128).T)
        bias_cols.append(sb3[s].reshape(MS, 128).T)
        m["bias"] = np.ascontiguousarray(np.concatenate(bias_cols, axis=1))
        m["sw1t"] = _pack_kblocks(sw1[s].T.copy()).astype(NPDT)
        m["sw3t"] = _pack_kblocks(sw3[s].T.copy()).astype(NPDT)
        m["sw2ot"] = _pack_kblocks(sw2[:, s].T @ ow.T).astype(NPDT)
        in_maps.append(m)

    # dense combine weights [B, E] (zero except the top-2 experts per token)
    combine_w = np.zeros((B, E), np.float32)
    rows = np.arange(B)
    combine_w[rows[:, None], order] = np.take_along_axis(scores, order, axis=1)
    # analytic bias terms: sum_e combine[:,e] * (b2[e] @ ow.T)  +  sb2 @ ow.T + ob
    base = combine_w @ (b2 @ ow.T) + sb2 @ ow.T + ob

    return dict(nc=nc, caps=caps, slot_exp=slot_exp, in_maps=in_maps,
                tok_lists=tok_lists, combine_w=combine_w, base=base)


def combine(p, results):
    """Combine per-core device partials into the full [B, OUT] output."""
    caps, slot_exp, tok_lists, combine_w = (
        p["caps"], p["slot_exp"], p["tok_lists"], p["combine_w"])
    out = p["base"].astype(np.float32).copy()
    for c in range(NCORES):
        r = results[c]
        out += r["zt"].astype(np.float32).T
        for j in range(EPC):
            e = slot_exp[c][j]
            toks = tok_lists[e]
            off = sum(caps[:j])
            yre = r["yr"][:, off:off + len(toks)].astype(np.float32)  # [OUT, cnt]
            out[toks] += combine_w[toks, e][:, None] * yre.T
    return out


def kernel(x, task_id, gate_w, w1, b1, w2, b2, w3, b3,
           sw1, sb1, sw2, sb2, sw3, sb3, ow, ob):
    global LAST_RESULTS
    p = prepare(x, task_id, gate_w, w1, b1, w2, b2, w3, b3,
                sw1, sb1, sw2, sb2, sw3, sb3, ow, ob)
    res = run_bass_kernel_spmd(
        p["nc"], p["in_maps"], core_ids=list(range(NCORES)),
        trace=TRACE, **TRACE_KW)
    LAST_RESULTS = res
    return combine(p, res.results)


# revision 37
# speedup vs baseline: 1.0203x; 1.0065x over previous
"""MoE routing kernel for 8 Trainium2 NeuronCores (Bass/Tile, SPMD).

Strategy (expert-parallel, matching the sharding hint):
  - Host computes the gate (softmax + top-2) and dispatches tokens: each of
    the 8 cores owns 2 of the 16 routed experts and receives only the tokens
    routed to its experts (gathered + transposed + zero-padded to a slot
    capacity).  Experts are paired hot+cold across cores so the two slot
    capacities (cap1 >= cap2) are as small as possible.
  - The output layer (ow) is linear and commutes with the weighted combine,
    so it is folded into each expert's second matmul on the host
    (w2ot = w2[e].T @ ow.T), shrinking stage-2 work by W/OUT = 4x.
  - The shared expert is sharded over its intermediate dim (2048/8=256 rows
    per core); every core computes a partial for all 2048 tokens, also with
    ow folded in.  Bias terms that commute with the output layer
    (b2, sb2, ob) are applied analytically on the host.
  - Schedule: the shared expert runs FIRST (its inputs are small, so the PE
    starts almost immediately) while the bulky routed-expert weights stream
    in behind it; the routed experts then run back-to-back.  All DMAs are
    emitted in first-use order across multiple queues.
  - Host combines: scatter-add of combine-weight-scaled routed partials +
    shared partials + analytic bias terms.
"""
import sys

if "/opt/trn_rl_repo" not in sys.path:
    sys.path.insert(0, "/opt/trn_rl_repo")

import numpy as np
import concourse.bass as bass
import concourse.tile as tile
from concourse import mybir
from concourse.bass_utils import run_bass_kernel_spmd

B = 2048
W = 512
E = 16
TOPK = 2
INTER = 1024
SH = 2048
OUT = 128
NCORES = 8
EPC = E // NCORES          # expert slots per core = 2
SHS = SH // NCORES         # shared-expert inter slice per core = 256
KW = W // 128              # k-tiles over W = 4
MI = INTER // 128          # m-tiles over INTER = 8
MS = SHS // 128            # m-tiles over shared slice = 2
CHUNKS = (512, 512, 512, 512)        # shared-expert token chunks (sum = B)
assert sum(CHUNKS) == B
F32 = mybir.dt.float32
F16 = mybir.dt.float16
DT = F16                   # device datapath dtype for matmul operands
NPDT = np.float16

# set by test.py to collect a profile; results stashed in LAST_RESULTS
TRACE = False
TRACE_KW = {}
LAST_RESULTS = None


def _legalize_waits(nc):
    """This container's walrus accepts at most 1 sync wait per instruction
    (2 for EventSemaphore).  Hoist excess waits emitted by the Tile
    scheduler into standalone EventSemaphore instructions."""
    for fn in nc.m.functions:
        for blk in fn.blocks:
            out = []
            changed = False
            for inst in blk.instructions:
                si = getattr(inst, "sync_info", None)
                waits = list(si.on_wait) if si is not None and si.on_wait else []
                cap = 2 if isinstance(inst, mybir.InstEventSemaphore) else 1
                if len(waits) > cap:
                    extra, keep = waits[:-cap], waits[-cap:]
                    for i in range(0, len(extra), 2):
                        out.append(mybir.InstEventSemaphore(
                            name=nc.get_next_instruction_name(),
                            engine=inst.engine,
                            ins=[], outs=[],
                            sync_info=mybir.SyncInfo(
                                on_wait=list(extra[i:i + 2]), on_update=[]),
                        ))
                    si.on_wait = keep
                    changed = True
                out.append(inst)
            if changed:
                blk.instructions = out


def _strip_end_clears(nc):
    """Drop the end-of-program semaphore RANGE_CLEAR sweep (~6.5us of
    serial semaphore resets) plus the post-clear drain/barrier round.  The
    clears only prepare device state for re-executing the same loaded NEFF,
    which never happens here (each kernel() call builds a fresh program).
    The DMA-completion waits and the first drain/barrier round are kept, so
    outputs are still guaranteed complete at program end."""
    blk = nc.m.functions[0].blocks[-1]
    isa_idx = [i for i, inst in enumerate(blk.instructions)
               if isinstance(inst, mybir.InstISA)]
    if isa_idx:
        cut = isa_idx[0]
        if cut > 0 and isinstance(blk.instructions[cut - 1], mybir.InstDrain):
            cut -= 1
        blk.instructions = blk.instructions[:cut]
    # The remaining per-engine Drains carry the kernel semaphore range and
    # are expanded by the backend into the per-semaphore reset sweep; drop
    # them too (their sync_info waits/updates move to a plain EventSemaphore
    # so the barrier chain stays intact).
    out = []
    for inst in blk.instructions:
        if isinstance(inst, mybir.InstDrain):
            si = getattr(inst, "sync_info", None)
            if si is not None and (si.on_wait or si.on_update):
                out.append(mybir.InstEventSemaphore(
                    name=nc.get_next_instruction_name(),
                    engine=inst.engine, ins=[], outs=[],
                    sync_info=mybir.SyncInfo(
                        on_wait=list(si.on_wait or []),
                        on_update=list(si.on_update or [])),
                ))
            continue
        out.append(inst)
    blk.instructions = out


def _merge_end_block(nc):
    """Append the (now tiny) end block's instructions to the body block and
    drop the separate block, removing the branch + its ~16KB end-of-program
    iram fetch from the critical path."""
    fn = nc.m.functions[0]
    if len(fn.blocks) < 3:
        return
    body, end = fn.blocks[-2], fn.blocks[-1]
    body.instructions = [
        inst for inst in body.instructions
        if not isinstance(inst, mybir.InstUnconditionalBranch)
    ] + list(end.instructions)
    fn.blocks.pop()


def _build_nc(caps, legalize=True):
    """Build the SPMD Bass program for per-slot token capacities
    `caps = (cap1, cap2)` (multiples of 16, each <= 512)."""
    nc = bass.Bass("TRN2", target_bir_lowering=False, debug=False)
    capsum = sum(caps)

    def din(name, f, dt=DT):
        return nc.dram_tensor(name, [128, f], dt, kind="ExternalInput").ap()

    # shared expert + all tokens (needed first)
    bias = din("bias", EPC * 2 * MI + 2 * MS, F32)  # b1/b3 per slot, sb1/sb3
    sw1t = din("sw1t", KW * SHS)             # shared slice: sw1[s].T packed k-blocks
    sw3t = din("sw3t", KW * SHS)
    sw2ot = din("sw2ot", MS * OUT)           # (sw2[:, s].T @ ow.T) packed
    xtc = din("xtc", B * KW)                 # x.T packed per chunk: chunk c at
    #   cols [c*KW*CH, (c+1)*KW*CH), k-block k at [c*KW*CH + k*CH, ... + CH)
    # routed expert slots
    w1s = [din(f"w1t{j}", KW * INTER) for j in range(EPC)]
    w3s = [din(f"w3t{j}", KW * INTER) for j in range(EPC)]
    xgs = [din(f"xg{j}", KW * caps[j]) for j in range(EPC)]
    w2s = [din(f"w2ot{j}", MI * OUT) for j in range(EPC)]

    yr = nc.dram_tensor("yr", [128, capsum], F16, kind="ExternalOutput").ap()
    zt = nc.dram_tensor("zt", [128, B], F16, kind="ExternalOutput").ap()

    LR = mybir.ActivationFunctionType.Lrelu

    with tile.TileContext(nc) as tc:
        with tc.tile_pool(name="wts", bufs=1) as wts, \
             tc.tile_pool(name="work", bufs=3) as work, \
             tc.tile_pool(name="hts", bufs=1) as hts, \
             tc.tile_pool(name="outs", bufs=2) as outs, \
             tc.tile_pool(name="ps", bufs=2, space="PSUM") as ps:

            # ---- DMAs in first-use order, spread across queues.  Tile
            # dependencies are whole-tile, so the first two chunks of x are
            # split into half-K tiles: the PE's first matmul only needs
            # sw1 (256KB) + the first half-chunk (256KB).  DMA triggers cost
            # ~600ns of sequencer time each, so pieces are kept >= 256KB.
            sw1_t = wts.tile([128, KW * SHS], DT, tag="sw1")
            nc.scalar.dma_start(sw1_t[:], sw1t[:])
            sw3_t = wts.tile([128, KW * SHS], DT, tag="sw3")
            nc.scalar.dma_start(sw3_t[:], sw3t[:])
            bias_t = wts.tile([128, bias.shape[1]], F32, tag="bias")
            nc.scalar.dma_start(bias_t[:], bias[:])
            sw2_t = wts.tile([128, MS * OUT], DT, tag="sw2")
            nc.scalar.dma_start(sw2_t[:], sw2ot[:])

            def sw3_ap(k, m):
                return sw3_t[:, (k * MS + m) * 128:(k * MS + m + 1) * 128]
            # xt chunk tiles; chunks 0 and 1 as two half-K tiles each
            xt_pieces = []   # per chunk: list of (tile, k_lo, k_hi)
            off = 0
            for c, ch in enumerate(CHUNKS):
                if c < 2:
                    pieces = []
                    for h, (klo, khi) in enumerate(((0, KW // 2), (KW // 2, KW))):
                        t = wts.tile([128, (khi - klo) * ch], DT,
                                     tag=f"xtc{c}h{h}", name=f"xtc{c}h{h}")
                        nc.sync.dma_start(
                            t[:], xtc[:, off * KW + klo * ch: off * KW + khi * ch])
                        pieces.append((t, klo, khi))
                    xt_pieces.append(pieces)
                else:
                    t = wts.tile([128, KW * ch], DT, tag=f"xtc{c}", name=f"xtc{c}")
                    nc.sync.dma_start(t[:], xtc[:, off * KW:(off + ch) * KW])
                    xt_pieces.append([(t, 0, KW)])
                off += ch

            def xt_ap(c, k, ch):
                for t, klo, khi in xt_pieces[c]:
                    if klo <= k < khi:
                        return t[:, (k - klo) * ch:(k - klo + 1) * ch]
                raise AssertionError
            # expert slots stream behind the shared phase
            w1_ts, w3_ts, xg_ts, w2_ts = [], [], [], []
            for j in range(EPC):
                eng = nc.scalar if j == 0 else nc.sync
                t = wts.tile([128, KW * INTER], DT, tag=f"w1s{j}")
                eng.dma_start(t[:], w1s[j][:])
                w1_ts.append(t)
                t = wts.tile([128, KW * caps[j]], DT, tag=f"xgs{j}")
                nc.gpsimd.dma_start(t[:], xgs[j][:])
                xg_ts.append(t)
                t = wts.tile([128, KW * INTER], DT, tag=f"w3s{j}")
                eng.dma_start(t[:], w3s[j][:])
                w3_ts.append(t)
                t = wts.tile([128, MI * OUT], DT, tag=f"w2s{j}")
                nc.gpsimd.dma_start(t[:], w2s[j][:])
                w2_ts.append(t)

            # ---- activation-table preload: a tiny LRELU on a memset tile so
            # the first real LEAKY_RELU doesn't pay ACT_TABLE_LOAD.  Emitted
            # after the DMA triggers so it doesn't delay them.
            warm = wts.tile([128, 1], F32, tag="warm")
            nc.gpsimd.memset(warm[:], 0.0)
            warm2 = wts.tile([128, 1], DT, tag="warm2")
            nc.scalar.activation(warm2[:], warm[:], LR, alpha=0.01)

            def b_ap(col):  # [128,1] per-partition bias column
                return bias_t[:, col:col + 1]

            def epilogue(p1, p3, bcol1, bcol3, csz, htag):
                """h = lrelu(p1 + b1) * (p3 + b3), split over 3 engines."""
                a = work.tile([128, csz], DT, tag="act_a")
                nc.scalar.activation(a[:], p1[:], LR, bias=b_ap(bcol1), alpha=0.01)
                t3 = work.tile([128, csz], DT, tag="act_b")
                nc.vector.tensor_scalar_add(t3[:], p3[:], b_ap(bcol3))
                ht = hts.tile([128, csz], DT, tag=htag, bufs=3)
                nc.vector.tensor_mul(ht[:], a[:], t3[:])
                return ht

            # ---- shared expert: token chunks, stage-2 trails by one chunk
            # so the in-order PE stream never waits on the DVE.
            pend = None

            def flush_shared(pz, t0, ch, hs_list):
                for m, hs in hs_list:
                    nc.tensor.matmul(pz[:], sw2_t[:, m * OUT:(m + 1) * OUT],
                                     hs[:], start=(m == 0), stop=(m == MS - 1))
                zo = outs.tile([128, ch], F16, tag="zo")
                nc.vector.tensor_copy(zo[:], pz[:])
                nc.sync.dma_start(zt[:, t0:t0 + ch], zo[:])

            t0 = 0
            for c, ch in enumerate(CHUNKS):
                pz = ps.tile([128, ch], F32, tag="py")
                hs_list = []
                if c == 0:
                    # first chunk: run BOTH m-tiles' p1 matmuls before any
                    # p3, so the PE has ~2us of sw1-only work while the sw3
                    # tile is still streaming in behind sw1 on its queue.
                    p1s, p3s = [], []
                    for m in range(MS):
                        p1 = ps.tile([128, ch], F32, tag="p1", bufs=3)
                        for k in range(KW):
                            nc.tensor.matmul(p1[:], sw1_t[:, (k * MS + m) * 128:(k * MS + m + 1) * 128],
                                             xt_ap(c, k, ch),
                                             start=(k == 0), stop=(k == KW - 1))
                        p1s.append(p1)
                    for m in range(MS):
                        p3 = ps.tile([128, ch], F32, tag="p3", bufs=3)
                        for k in range(KW):
                            nc.tensor.matmul(p3[:], sw3_ap(k, m), xt_ap(c, k, ch),
                                             start=(k == 0), stop=(k == KW - 1))
                        p3s.append(p3)
                    for m in range(MS):
                        hs = epilogue(p1s[m], p3s[m], EPC * 2 * MI + m,
                                      EPC * 2 * MI + MS + m, ch, f"hs{m}")
                        hs_list.append((m, hs))
                else:
                    for m in range(MS):
                        p1 = ps.tile([128, ch], F32, tag="p1", bufs=3)
                        p3 = ps.tile([128, ch], F32, tag="p3", bufs=3)
                        for k in range(KW):
                            nc.tensor.matmul(p1[:], sw1_t[:, (k * MS + m) * 128:(k * MS + m + 1) * 128],
                                             xt_ap(c, k, ch),
                                             start=(k == 0), stop=(k == KW - 1))
                        for k in range(KW):
                            nc.tensor.matmul(p3[:], sw3_ap(k, m), xt_ap(c, k, ch),
                                             start=(k == 0), stop=(k == KW - 1))
                        hs = epilogue(p1, p3, EPC * 2 * MI + m, EPC * 2 * MI + MS + m,
                                      ch, f"hs{m}")
                        hs_list.append((m, hs))
                if pend is not None:
                    flush_shared(*pend)
                pend = (pz, t0, ch, hs_list)
                t0 += ch
            # the last chunk's flush is deferred into the expert stream below
            # so the PE never waits on its DVE epilogue at the phase boundary.

            # ---- routed experts: stage-2 trails stage-1 by LAG m-tiles,
            # with the lag carried ACROSS slot boundaries so the PE stream
            # has no drain stall between experts.
            LAG = 2
            pend2 = []   # (j, py, m, ht)

            def flush_one():
                jd, pyd, md, htd = pend2.pop(0)
                nc.tensor.matmul(pyd[:], w2_ts[jd][:, md * OUT:(md + 1) * OUT],
                                 htd[:], start=(md == 0), stop=(md == MI - 1))
                if md == MI - 1:
                    capd = caps[jd]
                    yo = outs.tile([128, capd], F16, tag="yo")
                    nc.vector.tensor_copy(yo[:], pyd[:])
                    off = sum(caps[:jd])
                    nc.sync.dma_start(yr[:, off:off + capd], yo[:])

            for j in range(EPC):
                cap = caps[j]
                py = ps.tile([128, cap], F32, tag="py")
                for m in range(MI):
                    p1 = ps.tile([128, cap], F32, tag="p1", bufs=3)
                    p3 = ps.tile([128, cap], F32, tag="p3", bufs=3)
                    for k in range(KW):
                        nc.tensor.matmul(p1[:], w1_ts[j][:, (k * MI + m) * 128:(k * MI + m + 1) * 128],
                                         xg_ts[j][:, k * cap:(k + 1) * cap],
                                         start=(k == 0), stop=(k == KW - 1))
                    for k in range(KW):
                        nc.tensor.matmul(p3[:], w3_ts[j][:, (k * MI + m) * 128:(k * MI + m + 1) * 128],
                                         xg_ts[j][:, k * cap:(k + 1) * cap],
                                         start=(k == 0), stop=(k == KW - 1))
                    if pend is not None:
                        flush_shared(*pend)   # last shared chunk, one m-tile in
                        pend = None
                    ht = epilogue(p1, p3, j * 2 * MI + m, j * 2 * MI + MI + m,
                                  cap, f"ht{m}")
                    pend2.append((j, py, m, ht))
                    if len(pend2) > LAG:
                        flush_one()
            while pend2:
                flush_one()

    _strip_end_clears(nc)
    _merge_end_block(nc)
    if legalize:
        _legalize_waits(nc)
    return nc


_NC_CACHE = {}


def _pack_kblocks(mat):
    """[Ktot, F] -> [128, (Ktot/128)*F] with col block k = mat[128k:128(k+1), :]."""
    ktot, f = mat.shape
    assert ktot % 128 == 0
    return np.ascontiguousarray(
        mat.reshape(ktot // 128, 128, f).transpose(1, 0, 2).reshape(128, -1))


def _ceil8(n):
    return max(128, -(-n // 8) * 8)


def prepare(x, task_id, gate_w, w1, b1, w2, b2, w3, b3,
            sw1, sb1, sw2, sb2, sw3, sb3, ow, ob):
    """Host-side routing + packing.  Returns everything needed to launch the
    device program and combine its partial outputs."""
    x = np.asarray(x, np.float32)
    f32 = lambda a: np.asarray(a, np.float32)
    gate_w, w1, b1, w2, b2, w3, b3 = map(f32, (gate_w, w1, b1, w2, b2, w3, b3))
    sw1, sb1, sw2, sb2, sw3, sb3, ow, ob = map(f32, (sw1, sb1, sw2, sb2, sw3, sb3, ow, ob))

    # ---- host gate: softmax + top-2 (the routing decision) ----
    logits = x @ gate_w.T
    logits -= logits.max(axis=1, keepdims=True)
    ex = np.exp(logits)
    scores = ex / ex.sum(axis=1, keepdims=True)            # [B, E] fp32
    order = np.argsort(-scores, axis=1, kind="stable")[:, :TOPK]   # [B, 2]

    tok_lists = []
    for e in range(E):
        sel = np.nonzero((order == e).any(axis=1))[0]
        tok_lists.append(sel)

    # hot+cold pairing: core i gets (rank i, rank 15-i) by token count
    rank = sorted(range(E), key=lambda e: -len(tok_lists[e]))
    slot_exp = [[rank[i], rank[E - 1 - i]] for i in range(NCORES)]
    caps = tuple(_ceil8(max(len(tok_lists[slot_exp[c][j]]) for c in range(NCORES)))
                 for j in range(EPC))

    if caps not in _NC_CACHE:
        _NC_CACHE[caps] = _build_nc(caps)
    nc = _NC_CACHE[caps]

    # ---- pack per-core inputs (device datapath dtype) ----
    # xtc: chunk-major, then k-block: [128, B * KW]
    xt_k = x.T.reshape(KW, 128, B)                         # [k, p, t]
    blocks = []
    off = 0
    for ch in CHUNKS:
        for k in range(KW):
            blocks.append(xt_k[k, :, off:off + ch])
        off += ch
    xtc = np.ascontiguousarray(np.concatenate(blocks, axis=1)).astype(NPDT)
    in_maps = []
    for c in range(NCORES):
        m = {"xtc": xtc}
        bias_cols = []
        for j in range(EPC):
            e = slot_exp[c][j]
            cap = caps[j]
            toks = tok_lists[e]
            xge = np.zeros((W, cap), np.float32)
            xge[:, :len(toks)] = x[toks].T
            m[f"xg{j}"] = _pack_kblocks(xge).astype(NPDT)
            m[f"w1t{j}"] = _pack_kblocks(w1[e].T.copy()).astype(NPDT)
            m[f"w3t{j}"] = _pack_kblocks(w3[e].T.copy()).astype(NPDT)
            m[f"w2ot{j}"] = _pack_kblocks(w2[e].T @ ow.T).astype(NPDT)
        for j in range(EPC):
            e = slot_exp[c][j]
            bias_cols.append(b1[e].reshape(MI, 128).T)     # [128, MI]
            bias_cols.append(b3[e].reshape(MI, 128).T)
        s = slice(c * SHS, (c + 1) * SHS)
        bias_cols.append(sb1[s].reshape(MS, 128).T)
        bias_cols.append(sb3[s].reshape(MS, 128).T)
        m["bias"] = np.ascontiguousarray(np.concatenate(bias_cols, axis=1))
        m["sw1t"] = _pack_kblocks(sw1[s].T.copy()).astype(NPDT)
        m["sw3t"] = _pack_kblocks(sw3[s].T.copy()).astype(NPDT)
        m["sw2ot"] = _pack_kblocks(sw2[:, s].T @ ow.T).astype(NPDT)
        in_maps.append(m)

    # dense combine weights [B, E] (zero except the top-2 experts per token)
    combine_w = np.zeros((B, E), np.float32)
    rows = np.arange(B)
    combine_w[rows[:, None], order] = np.take_along_axis(scores, order, axis=1)
    # analytic bias terms: sum_e combine[:,e] * (b2[e] @ ow.T)  +  sb2 @ ow.T + ob
    base = combine_w @ (b2 @ ow.T) + sb2 @ ow.T + ob

    return dict(nc=nc, caps=caps, slot_exp=slot_exp, in_maps=in_maps,
                tok_lists=tok_lists, combine_w=combine_w, base=base)


def combine(p, results):
    """Combine per-core device partials into the full [B, OUT] output."""
    caps, slot_exp, tok_lists, combine_w = (
        p["caps"], p["slot_exp"], p["tok_lists"], p["combine_w"])
    out = p["base"].astype(np.float32).copy()
    for c in range(NCORES):
        r = results[c]
        out += r["zt"].astype(np.float32).T
        for j in range(EPC):
            e = slot_exp[c][j]
            toks = tok_lists[e]
            off = sum(caps[:j])
            yre = r["yr"][:, off:off + len(toks)].astype(np.float32)  # [OUT, cnt]
            out[toks] += combine_w[toks, e][:, None] * yre.T
    return out


def kernel(x, task_id, gate_w, w1, b1, w2, b2, w3, b3,
           sw1, sb1, sw2, sb2, sw3, sb3, ow, ob):
    global LAST_RESULTS
    p = prepare(x, task_id, gate_w, w1, b1, w2, b2, w3, b3,
                sw1, sb1, sw2, sb2, sw3, sb3, ow, ob)
    res = run_bass_kernel_spmd(
        p["nc"], p["in_maps"], core_ids=list(range(NCORES)),
        trace=TRACE, **TRACE_KW)
    LAST_RESULTS = res
    return combine(p, res.results)


# revision 38
# speedup vs baseline: 1.0392x; 1.0185x over previous
"""MoE routing kernel for 8 Trainium2 NeuronCores (Bass/Tile, SPMD).

Strategy (expert-parallel, matching the sharding hint):
  - Host computes the gate (softmax + top-2) and dispatches tokens: each of
    the 8 cores owns 2 of the 16 routed experts and receives only the tokens
    routed to its experts (gathered + transposed + zero-padded to a slot
    capacity).  Experts are paired hot+cold across cores so the two slot
    capacities (cap1 >= cap2) are as small as possible.
  - The output layer (ow) is linear and commutes with the weighted combine,
    so it is folded into each expert's second matmul on the host
    (w2ot = w2[e].T @ ow.T), shrinking stage-2 work by W/OUT = 4x.
  - The shared expert is sharded over its intermediate dim (2048/8=256 rows
    per core); every core computes a partial for all 2048 tokens, also with
    ow folded in.  Bias terms that commute with the output layer
    (b2, sb2, ob) are applied analytically on the host.
  - Schedule: the shared expert runs FIRST (its inputs are small, so the PE
    starts almost immediately) while the bulky routed-expert weights stream
    in behind it; the routed experts then run back-to-back.  All DMAs are
    emitted in first-use order across multiple queues.
  - Host combines: scatter-add of combine-weight-scaled routed partials +
    shared partials + analytic bias terms.
"""
import sys

if "/opt/trn_rl_repo" not in sys.path:
    sys.path.insert(0, "/opt/trn_rl_repo")

import numpy as np
import concourse.bass as bass
import concourse.tile as tile
from concourse import mybir
from concourse.bass_utils import run_bass_kernel_spmd

B = 2048
W = 512
E = 16
TOPK = 2
INTER = 1024
SH = 2048
OUT = 128
NCORES = 8
EPC = E // NCORES          # expert slots per core = 2
SHS = SH // NCORES         # shared-expert inter slice per core = 256
KW = W // 128              # k-tiles over W = 4
MI = INTER // 128          # m-tiles over INTER = 8
MS = SHS // 128            # m-tiles over shared slice = 2
CHUNKS = (512, 512, 512, 512)        # shared-expert token chunks (sum = B)
assert sum(CHUNKS) == B
F32 = mybir.dt.float32
F16 = mybir.dt.float16
DT = F16                   # device datapath dtype for matmul operands
NPDT = np.float16

# set by test.py to collect a profile; results stashed in LAST_RESULTS
TRACE = False
TRACE_KW = {}
LAST_RESULTS = None


def _legalize_waits(nc):
    """This container's walrus accepts at most 1 sync wait per instruction
    (2 for EventSemaphore).  Hoist excess waits emitted by the Tile
    scheduler into standalone EventSemaphore instructions."""
    for fn in nc.m.functions:
        for blk in fn.blocks:
            out = []
            changed = False
            for inst in blk.instructions:
                si = getattr(inst, "sync_info", None)
                waits = list(si.on_wait) if si is not None and si.on_wait else []
                cap = 2 if isinstance(inst, mybir.InstEventSemaphore) else 1
                if len(waits) > cap:
                    extra, keep = waits[:-cap], waits[-cap:]
                    for i in range(0, len(extra), 2):
                        out.append(mybir.InstEventSemaphore(
                            name=nc.get_next_instruction_name(),
                            engine=inst.engine,
                            ins=[], outs=[],
                            sync_info=mybir.SyncInfo(
                                on_wait=list(extra[i:i + 2]), on_update=[]),
                        ))
                    si.on_wait = keep
                    changed = True
                out.append(inst)
            if changed:
                blk.instructions = out


def _strip_end_clears(nc):
    """Drop the end-of-program semaphore RANGE_CLEAR sweep (~6.5us of
    serial semaphore resets) plus the post-clear drain/barrier round.  The
    clears only prepare device state for re-executing the same loaded NEFF,
    which never happens here (each kernel() call builds a fresh program).
    The DMA-completion waits and the first drain/barrier round are kept, so
    outputs are still guaranteed complete at program end."""
    blk = nc.m.functions[0].blocks[-1]
    isa_idx = [i for i, inst in enumerate(blk.instructions)
               if isinstance(inst, mybir.InstISA)]
    if isa_idx:
        cut = isa_idx[0]
        if cut > 0 and isinstance(blk.instructions[cut - 1], mybir.InstDrain):
            cut -= 1
        blk.instructions = blk.instructions[:cut]
    # The remaining per-engine Drains carry the kernel semaphore range and
    # are expanded by the backend into the per-semaphore reset sweep; drop
    # them too (their sync_info waits/updates move to a plain EventSemaphore
    # so the barrier chain stays intact).
    out = []
    for inst in blk.instructions:
        if isinstance(inst, mybir.InstDrain):
            si = getattr(inst, "sync_info", None)
            if si is not None and (si.on_wait or si.on_update):
                out.append(mybir.InstEventSemaphore(
                    name=nc.get_next_instruction_name(),
                    engine=inst.engine, ins=[], outs=[],
                    sync_info=mybir.SyncInfo(
                        on_wait=list(si.on_wait or []),
                        on_update=list(si.on_update or [])),
                ))
            continue
        out.append(inst)
    blk.instructions = out


def _merge_end_block(nc):
    """Append the (now tiny) end block's instructions to the body block and
    drop the separate block, removing the branch + its ~16KB end-of-program
    iram fetch from the critical path."""
    fn = nc.m.functions[0]
    if len(fn.blocks) < 3:
        return
    body, end = fn.blocks[-2], fn.blocks[-1]
    body.instructions = [
        inst for inst in body.instructions
        if not isinstance(inst, mybir.InstUnconditionalBranch)
    ] + list(end.instructions)
    fn.blocks.pop()


def _build_nc(caps, legalize=True):
    """Build the SPMD Bass program for per-slot token capacities
    `caps = (cap1, cap2)` (multiples of 16, each <= 512)."""
    nc = bass.Bass("TRN2", target_bir_lowering=False, debug=False)
    capsum = sum(caps)

    def din(name, f, dt=DT):
        return nc.dram_tensor(name, [128, f], dt, kind="ExternalInput").ap()

    # shared expert + all tokens (needed first)
    bias = din("bias", EPC * 2 * MI + 2 * MS, F32)  # b1/b3 per slot, sb1/sb3
    sw1t = din("sw1t", KW * SHS)             # shared slice: sw1[s].T packed k-blocks
    sw3t = din("sw3t", KW * SHS)
    sw2ot = din("sw2ot", MS * OUT)           # (sw2[:, s].T @ ow.T) packed
    xtc = din("xtc", B * KW)                 # x.T packed per chunk: chunk c at
    #   cols [c*KW*CH, (c+1)*KW*CH), k-block k at [c*KW*CH + k*CH, ... + CH)
    # routed expert slots
    w1s = [din(f"w1t{j}", KW * INTER) for j in range(EPC)]
    w3s = [din(f"w3t{j}", KW * INTER) for j in range(EPC)]
    xgs = [din(f"xg{j}", KW * caps[j]) for j in range(EPC)]
    w2s = [din(f"w2ot{j}", MI * OUT) for j in range(EPC)]

    yr = nc.dram_tensor("yr", [128, capsum], F16, kind="ExternalOutput").ap()
    zt = nc.dram_tensor("zt", [128, B], F16, kind="ExternalOutput").ap()

    LR = mybir.ActivationFunctionType.Lrelu
    IDT = mybir.ActivationFunctionType.Identity

    with tile.TileContext(nc) as tc:
        with tc.tile_pool(name="wts", bufs=1) as wts, \
             tc.tile_pool(name="work", bufs=3) as work, \
             tc.tile_pool(name="hts", bufs=1) as hts, \
             tc.tile_pool(name="outs", bufs=2) as outs, \
             tc.tile_pool(name="ps", bufs=2, space="PSUM") as ps:

            # ---- DMAs in first-use order, spread across queues.  Tile
            # dependencies are whole-tile, so the first two chunks of x are
            # split into half-K tiles: the PE's first matmul only needs
            # sw1 (256KB) + the first half-chunk (256KB).  DMA triggers cost
            # ~600ns of sequencer time each, so pieces are kept >= 256KB.
            sw1_t = wts.tile([128, KW * SHS], DT, tag="sw1")
            nc.scalar.dma_start(sw1_t[:], sw1t[:])
            sw3_t = wts.tile([128, KW * SHS], DT, tag="sw3")
            nc.scalar.dma_start(sw3_t[:], sw3t[:])
            bias_t = wts.tile([128, bias.shape[1]], F32, tag="bias")
            nc.scalar.dma_start(bias_t[:], bias[:])
            sw2_t = wts.tile([128, MS * OUT], DT, tag="sw2")
            nc.scalar.dma_start(sw2_t[:], sw2ot[:])

            def sw3_ap(k, m):
                return sw3_t[:, (k * MS + m) * 128:(k * MS + m + 1) * 128]
            # xt chunk tiles; chunks 0 and 1 as two half-K tiles each
            xt_pieces = []   # per chunk: list of (tile, k_lo, k_hi)
            off = 0
            for c, ch in enumerate(CHUNKS):
                if c < 2:
                    pieces = []
                    for h, (klo, khi) in enumerate(((0, KW // 2), (KW // 2, KW))):
                        t = wts.tile([128, (khi - klo) * ch], DT,
                                     tag=f"xtc{c}h{h}", name=f"xtc{c}h{h}")
                        nc.sync.dma_start(
                            t[:], xtc[:, off * KW + klo * ch: off * KW + khi * ch])
                        pieces.append((t, klo, khi))
                    xt_pieces.append(pieces)
                else:
                    t = wts.tile([128, KW * ch], DT, tag=f"xtc{c}", name=f"xtc{c}")
                    nc.sync.dma_start(t[:], xtc[:, off * KW:(off + ch) * KW])
                    xt_pieces.append([(t, 0, KW)])
                off += ch

            def xt_ap(c, k, ch):
                for t, klo, khi in xt_pieces[c]:
                    if klo <= k < khi:
                        return t[:, (k - klo) * ch:(k - klo + 1) * ch]
                raise AssertionError
            # expert slots stream behind the shared phase
            w1_ts, w3_ts, xg_ts, w2_ts = [], [], [], []
            for j in range(EPC):
                eng = nc.scalar if j == 0 else nc.sync
                t = wts.tile([128, KW * INTER], DT, tag=f"w1s{j}")
                eng.dma_start(t[:], w1s[j][:])
                w1_ts.append(t)
                t = wts.tile([128, KW * caps[j]], DT, tag=f"xgs{j}")
                nc.gpsimd.dma_start(t[:], xgs[j][:])
                xg_ts.append(t)
                t = wts.tile([128, KW * INTER], DT, tag=f"w3s{j}")
                eng.dma_start(t[:], w3s[j][:])
                w3_ts.append(t)
                t = wts.tile([128, MI * OUT], DT, tag=f"w2s{j}")
                nc.gpsimd.dma_start(t[:], w2s[j][:])
                w2_ts.append(t)

            # ---- activation-table preload: a tiny LRELU on a memset tile so
            # the first real LEAKY_RELU doesn't pay ACT_TABLE_LOAD.  Emitted
            # after the DMA triggers so it doesn't delay them.
            warm = wts.tile([128, 1], F32, tag="warm")
            nc.gpsimd.memset(warm[:], 0.0)
            warm2 = wts.tile([128, 1], DT, tag="warm2")
            nc.scalar.activation(warm2[:], warm[:], LR, alpha=0.01)

            def b_ap(col):  # [128,1] per-partition bias column
                return bias_t[:, col:col + 1]

            def epilogue(p1, p3, bcol1, bcol3, csz, htag):
                """h = lrelu(p1 + b1) * (p3 + b3), split over 3 engines."""
                a = work.tile([128, csz], DT, tag="act_a")
                nc.scalar.activation(a[:], p1[:], LR, bias=b_ap(bcol1), alpha=0.01)
                t3 = work.tile([128, csz], DT, tag="act_b")
                nc.vector.tensor_scalar_add(t3[:], p3[:], b_ap(bcol3))
                ht = hts.tile([128, csz], DT, tag=htag, bufs=3)
                nc.vector.tensor_mul(ht[:], a[:], t3[:])
                return ht

            # ---- shared expert: token chunks, stage-2 trails by one chunk
            # so the in-order PE stream never waits on the DVE.
            pend = None

            def flush_shared(pz, t0, ch, hs_list):
                for m, hs in hs_list:
                    nc.tensor.matmul(pz[:], sw2_t[:, m * OUT:(m + 1) * OUT],
                                     hs[:], start=(m == 0), stop=(m == MS - 1))
                zo = outs.tile([128, ch], F16, tag="zo")
                nc.scalar.activation(zo[:], pz[:], IDT)
                nc.sync.dma_start(zt[:, t0:t0 + ch], zo[:])

            t0 = 0
            for c, ch in enumerate(CHUNKS):
                pz = ps.tile([128, ch], F32, tag="py")
                hs_list = []
                if c == 0:
                    # first chunk: run BOTH m-tiles' p1 matmuls before any
                    # p3, so the PE has ~2us of sw1-only work while the sw3
                    # tile is still streaming in behind sw1 on its queue.
                    p1s, p3s = [], []
                    for m in range(MS):
                        p1 = ps.tile([128, ch], F32, tag="p1", bufs=3)
                        for k in range(KW):
                            nc.tensor.matmul(p1[:], sw1_t[:, (k * MS + m) * 128:(k * MS + m + 1) * 128],
                                             xt_ap(c, k, ch),
                                             start=(k == 0), stop=(k == KW - 1))
                        p1s.append(p1)
                    for m in range(MS):
                        p3 = ps.tile([128, ch], F32, tag="p3", bufs=3)
                        for k in range(KW):
                            nc.tensor.matmul(p3[:], sw3_ap(k, m), xt_ap(c, k, ch),
                                             start=(k == 0), stop=(k == KW - 1))
                        p3s.append(p3)
                    for m in range(MS):
                        hs = epilogue(p1s[m], p3s[m], EPC * 2 * MI + m,
                                      EPC * 2 * MI + MS + m, ch, f"hs{m}")
                        hs_list.append((m, hs))
                else:
                    for m in range(MS):
                        p1 = ps.tile([128, ch], F32, tag="p1", bufs=3)
                        p3 = ps.tile([128, ch], F32, tag="p3", bufs=3)
                        for k in range(KW):
                            nc.tensor.matmul(p1[:], sw1_t[:, (k * MS + m) * 128:(k * MS + m + 1) * 128],
                                             xt_ap(c, k, ch),
                                             start=(k == 0), stop=(k == KW - 1))
                        for k in range(KW):
                            nc.tensor.matmul(p3[:], sw3_ap(k, m), xt_ap(c, k, ch),
                                             start=(k == 0), stop=(k == KW - 1))
                        hs = epilogue(p1, p3, EPC * 2 * MI + m, EPC * 2 * MI + MS + m,
                                      ch, f"hs{m}")
                        hs_list.append((m, hs))
                if pend is not None:
                    flush_shared(*pend)
                pend = (pz, t0, ch, hs_list)
                t0 += ch
            # the last chunk's flush is deferred into the expert stream below
            # so the PE never waits on its DVE epilogue at the phase boundary.

            # ---- routed experts: stage-2 trails stage-1 by LAG m-tiles,
            # with the lag carried ACROSS slot boundaries so the PE stream
            # has no drain stall between experts.
            LAG = 2
            pend2 = []   # (j, py, m, ht)

            def flush_one():
                jd, pyd, md, htd = pend2.pop(0)
                nc.tensor.matmul(pyd[:], w2_ts[jd][:, md * OUT:(md + 1) * OUT],
                                 htd[:], start=(md == 0), stop=(md == MI - 1))
                if md == MI - 1:
                    capd = caps[jd]
                    yo = outs.tile([128, capd], F16, tag="yo")
                    nc.scalar.activation(yo[:], pyd[:], IDT)
                    off = sum(caps[:jd])
                    nc.sync.dma_start(yr[:, off:off + capd], yo[:])

            for j in range(EPC):
                cap = caps[j]
                py = ps.tile([128, cap], F32, tag="py")
                for m in range(MI):
                    p1 = ps.tile([128, cap], F32, tag="p1", bufs=3)
                    p3 = ps.tile([128, cap], F32, tag="p3", bufs=3)
                    for k in range(KW):
                        nc.tensor.matmul(p1[:], w1_ts[j][:, (k * MI + m) * 128:(k * MI + m + 1) * 128],
                                         xg_ts[j][:, k * cap:(k + 1) * cap],
                                         start=(k == 0), stop=(k == KW - 1))
                    for k in range(KW):
                        nc.tensor.matmul(p3[:], w3_ts[j][:, (k * MI + m) * 128:(k * MI + m + 1) * 128],
                                         xg_ts[j][:, k * cap:(k + 1) * cap],
                                         start=(k == 0), stop=(k == KW - 1))
                    if pend is not None:
                        flush_shared(*pend)   # last shared chunk, one m-tile in
                        pend = None
                    ht = epilogue(p1, p3, j * 2 * MI + m, j * 2 * MI + MI + m,
                                  cap, f"ht{m}")
                    pend2.append((j, py, m, ht))
                    if len(pend2) > LAG:
                        flush_one()
            while pend2:
                flush_one()

    _strip_end_clears(nc)
    _merge_end_block(nc)
    if legalize:
        _legalize_waits(nc)
    return nc


_NC_CACHE = {}


def _pack_kblocks(mat):
    """[Ktot, F] -> [128, (Ktot/128)*F] with col block k = mat[128k:128(k+1), :]."""
    ktot, f = mat.shape
    assert ktot % 128 == 0
    return np.ascontiguousarray(
        mat.reshape(ktot // 128, 128, f).transpose(1, 0, 2).reshape(128, -1))


def _ceil8(n):
    return max(128, -(-n // 8) * 8)


def prepare(x, task_id, gate_w, w1, b1, w2, b2, w3, b3,
            sw1, sb1, sw2, sb2, sw3, sb3, ow, ob):
    """Host-side routing + packing.  Returns everything needed to launch the
    device program and combine its partial outputs."""
    x = np.asarray(x, np.float32)
    f32 = lambda a: np.asarray(a, np.float32)
    gate_w, w1, b1, w2, b2, w3, b3 = map(f32, (gate_w, w1, b1, w2, b2, w3, b3))
    sw1, sb1, sw2, sb2, sw3, sb3, ow, ob = map(f32, (sw1, sb1, sw2, sb2, sw3, sb3, ow, ob))

    # ---- host gate: softmax + top-2 (the routing decision) ----
    logits = x @ gate_w.T
    logits -= logits.max(axis=1, keepdims=True)
    ex = np.exp(logits)
    scores = ex / ex.sum(axis=1, keepdims=True)            # [B, E] fp32
    order = np.argsort(-scores, axis=1, kind="stable")[:, :TOPK]   # [B, 2]

    tok_lists = []
    for e in range(E):
        sel = np.nonzero((order == e).any(axis=1))[0]
        tok_lists.append(sel)

    # hot+cold pairing: core i gets (rank i, rank 15-i) by token count
    rank = sorted(range(E), key=lambda e: -len(tok_lists[e]))
    slot_exp = [[rank[i], rank[E - 1 - i]] for i in range(NCORES)]
    caps = tuple(_ceil8(max(len(tok_lists[slot_exp[c][j]]) for c in range(NCORES)))
                 for j in range(EPC))

    if caps not in _NC_CACHE:
        _NC_CACHE[caps] = _build_nc(caps)
    nc = _NC_CACHE[caps]

    # ---- pack per-core inputs (device datapath dtype) ----
    # xtc: chunk-major, then k-block: [128, B * KW]
    xt_k = x.T.reshape(KW, 128, B)                         # [k, p, t]
    blocks = []
    off = 0
    for ch in CHUNKS:
        for k in range(KW):
            blocks.append(xt_k[k, :, off:off + ch])
        off += ch
    xtc = np.ascontiguousarray(np.concatenate(blocks, axis=1)).astype(NPDT)
    in_maps = []
    for c in range(NCORES):
        m = {"xtc": xtc}
        bias_cols = []
        for j in range(EPC):
            e = slot_exp[c][j]
            cap = caps[j]
            toks = tok_lists[e]
            xge = np.zeros((W, cap), np.float32)
            xge[:, :len(toks)] = x[toks].T
            m[f"xg{j}"] = _pack_kblocks(xge).astype(NPDT)
            m[f"w1t{j}"] = _pack_kblocks(w1[e].T.copy()).astype(NPDT)
            m[f"w3t{j}"] = _pack_kblocks(w3[e].T.copy()).astype(NPDT)
            m[f"w2ot{j}"] = _pack_kblocks(w2[e].T @ ow.T).astype(NPDT)
        for j in range(EPC):
            e = slot_exp[c][j]
            bias_cols.append(b1[e].reshape(MI, 128).T)     # [128, MI]
            bias_cols.append(b3[e].reshape(MI, 128).T)
        s = slice(c * SHS, (c + 1) * SHS)
        bias_cols.append(sb1[s].reshape(MS, 128).T)
        bias_cols.append(sb3[s].reshape(MS, 128).T)
        m["bias"] = np.ascontiguousarray(np.concatenate(bias_cols, axis=1))
        m["sw1t"] = _pack_kblocks(sw1[s].T.copy()).astype(NPDT)
        m["sw3t"] = _pack_kblocks(sw3[s].T.copy()).astype(NPDT)
        m["sw2ot"] = _pack_kblocks(sw2[:, s].T @ ow.T).astype(NPDT)
        in_maps.append(m)

    # dense combine weights [B, E] (zero except the top-2 experts per token)
    combine_w = np.zeros((B, E), np.float32)
    rows = np.arange(B)
    combine_w[rows[:, None], order] = np.take_along_axis(scores, order, axis=1)
    # analytic bias terms: sum_e combine[:,e] * (b2[e] @ ow.T)  +  sb2 @ ow.T + ob
    base = combine_w @ (b2 @ ow.T) + sb2 @ ow.T + ob

    return dict(nc=nc, caps=caps, slot_exp=slot_exp, in_maps=in_maps,
                tok_lists=tok_lists, combine_w=combine_w, base=base)


def combine(p, results):
    """Combine per-core device partials into the full [B, OUT] output."""
    caps, slot_exp, tok_lists, combine_w = (
        p["caps"], p["slot_exp"], p["tok_lists"], p["combine_w"])
    out = p["base"].astype(np.float32).copy()
    for c in range(NCORES):
        r = results[c]
        out += r["zt"].astype(np.float32).T
        for j in range(EPC):
            e = slot_exp[c][j]
            toks = tok_lists[e]
            off = sum(caps[:j])
            yre = r["yr"][:, off:off + len(toks)].astype(np.float32)  # [OUT, cnt]
            out[toks] += combine_w[toks, e][:, None] * yre.T
    return out


def kernel(x, task_id, gate_w, w1, b1, w2, b2, w3, b3,
           sw1, sb1, sw2, sb2, sw3, sb3, ow, ob):
    global LAST_RESULTS
    p = prepare(x, task_id, gate_w, w1, b1, w2, b2, w3, b3,
                sw1, sb1, sw2, sb2, sw3, sb3, ow, ob)
    res = run_bass_kernel_spmd(
        p["nc"], p["in_maps"], core_ids=list(range(NCORES)),
        trace=TRACE, **TRACE_KW)
    LAST_RESULTS = res
    return combine(p, res.results)
